# revision 39
# baseline (speedup 1.0000x reference)
"""Trainium2 Bass kernel for BlockAttnRes.compute_all_inputs (bf16 pipeline).

Proven v2: 314880 ns, rel err 3.28e-03. Per-batch (5 rows) pipeline,
SWDGE loads, fold-transposes, bf16 I/O end-to-end.
"""

import numpy as np
import ml_dtypes

import concourse.bass as bass
import concourse.bacc as bacc
import concourse.mybir as mybir
from concourse import tile
from concourse.alu_op_type import AluOpType
from concourse.bass_utils import run_bass_kernel_spmd

L = 24
D = 2048
NUM_BLOCKS = 8
EPS = 1e-6
B, T = 2, 1024
N_CORES = 8

ROWS_PER_CORE = (B * T) // N_CORES  # 256
R = 5             # rows per batch
NJ = 25           # raw vectors per row: emb + 24 layer outputs
NS = 25           # sources per row
P = NJ * R        # 125 partitions per batch
NCHUNK = D // 128  # 16 d-chunks
CW = 152          # vt_sb per-chunk pitch: 128 (VT + 3 zero) + 24 qwT
NEG = -1e30

f32 = mybir.dt.float32
bf16 = mybir.dt.bfloat16
BF = ml_dtypes.bfloat16


def _source_matrix():
    M = np.zeros((NS, NJ), dtype=np.float32)
    M[0, 0] = 1.0
    for k in range(NUM_BLOCKS):
        for i in range(3):
            M[1 + 3 * k + i, 1 + 3 * k : 1 + 3 * k + i + 1] = 1.0
    return M


def _valid_matrix():
    V = np.zeros((L, NS), dtype=bool)
    for l in range(L):
        kb, ii = l // 3, l % 3
        V[l, 0] = True
        for k in range(kb):
            V[l, 3 * k + 3] = True
        if ii > 0:
            V[l, 3 * kb + ii] = True
    return V


def _build_consts(queries, key_norm_weight):
    M = _source_matrix()
    valid = _valid_matrix()
    eye_r = np.eye(R, dtype=np.float32)

    qw = (queries * key_norm_weight[None, :]).astype(np.float32)  # [L, D]
    qwT = np.ascontiguousarray(
        qw.reshape(L, NCHUNK, 128).transpose(2, 1, 0).reshape(128, NCHUNK * L)
    ).astype(BF)

    mtbd = np.einsum("nj,ab->ajbn", M, eye_r).reshape(P, NS * R)
    mtbd128 = np.zeros((P, 128), np.float32)
    mtbd128[:, :P] = mtbd
    mtbd128 = mtbd128.astype(BF)
    mbd = np.einsum("nj,ab->anbj", M, eye_r).reshape(NS * R, P)
    mbd128 = np.zeros((P, 128), np.float32)
    mbd128[:, :P] = mbd
    mbd128 = mbd128.astype(BF)
    eye_bd = np.zeros((P, 128), np.float32)
    eye_bd[:, :P] = np.eye(P, dtype=np.float32)
    diagm = np.einsum("ab,nl->anbl", eye_r, np.ones((NS, L), np.float32))
    diagm = np.ascontiguousarray(diagm.reshape(P, R * L)).astype(np.float32)
    maskneg = np.where(valid[:, None, :], 0.0, NEG)
    maskneg = np.broadcast_to(maskneg, (L, R, NS)).reshape(L, R * NS)
    maskneg = np.ascontiguousarray(maskneg).astype(np.float32)

    ident = np.eye(128, dtype=np.float32)
    return dict(qwT=qwT, mtbd=mtbd128, mbd=mbd128, eyebd=eye_bd, diagm=diagm,
                maskneg=maskneg, ident=ident)


def _batch_starts():
    starts = [R * b for b in range(ROWS_PER_CORE // R)]  # 0..250
    if starts[-1] + R < ROWS_PER_CORE:
        starts.append(ROWS_PER_CORE - R)  # 251 (overlaps; identical rewrites)
    return starts


def build_kernel(do_compile=True):
    nc = bacc.Bacc("TRN2", target_bir_lowering=False, debug=False)

    loT = nc.dram_tensor("loT", [ROWS_PER_CORE * NJ, D], bf16,
                         kind="ExternalInput").ap()
    qwT_d = nc.dram_tensor("qwT", [128, NCHUNK * L], bf16, kind="ExternalInput").ap()
    mtbd_d = nc.dram_tensor("mtbd", [P, 128], bf16, kind="ExternalInput").ap()
    mbd_d = nc.dram_tensor("mbd", [P, 128], bf16, kind="ExternalInput").ap()
    eyebd_d = nc.dram_tensor("eyebd", [P, 128], f32, kind="ExternalInput").ap()
    diagm_d = nc.dram_tensor("diagm", [P, R * L], f32, kind="ExternalInput").ap()
    maskneg_d = nc.dram_tensor("maskneg", [L, R * NS], f32, kind="ExternalInput").ap()
    ident_d = nc.dram_tensor("ident", [128, 128], f32, kind="ExternalInput").ap()
    outT = nc.dram_tensor("outT", [ROWS_PER_CORE * L, D], bf16,
                          kind="ExternalOutput").ap()

    with tile.TileContext(nc) as tc:
        with (
            tc.tile_pool(name="const", bufs=1) as const,
            tc.tile_pool(name="xpool", bufs=6) as xpool,
            tc.tile_pool(name="vtpool", bufs=3) as vtpool,
            tc.tile_pool(name="hpool", bufs=4) as hpool,
            tc.tile_pool(name="small", bufs=2) as small,
            tc.tile_pool(name="ps_ft", bufs=2, space=bass.MemorySpace.PSUM) as ps_ft,
            tc.tile_pool(name="ps_sc", bufs=2, space=bass.MemorySpace.PSUM) as ps_sc,
            tc.tile_pool(name="ps_sm", bufs=2, space=bass.MemorySpace.PSUM) as ps_sm,
            tc.tile_pool(name="ps_h", bufs=2, space=bass.MemorySpace.PSUM) as ps_h,
        ):
            qwT = const.tile([128, NCHUNK * L], bf16)
            nc.sync.dma_start(qwT[:], qwT_d[:])
            mtbd = const.tile([P, 128], bf16)
            nc.sync.dma_start(mtbd[:], mtbd_d[:])
            mbd = const.tile([P, 128], bf16)
            nc.sync.dma_start(mbd[:], mbd_d[:])
            eyebd = const.tile([P, 128], f32)
            nc.sync.dma_start(eyebd[:], eyebd_d[:])
            diagm = const.tile([P, R * L], f32)
            nc.sync.dma_start(diagm[:], diagm_d[:])
            maskneg = const.tile([L, R * NS], f32)
            nc.sync.dma_start(maskneg[:], maskneg_d[:])
            ident = const.tile([128, 128], f32)
            nc.sync.dma_start(ident[:], ident_d[:])
            epsb = const.tile([P, 1], f32)
            nc.vector.memset(epsb[:], EPS)

            b_idx = 0
            for row0 in _batch_starts():
                X = xpool.tile([P, D], bf16)
                nc.gpsimd.dma_start(X[:], loT[row0 * NJ : row0 * NJ + P, :])

                vt_sb = vtpool.tile([128, NCHUNK * CW], bf16)
                vt3 = vt_sb.rearrange("p (c w) -> p c w", w=CW)
                if b_idx < 3:  # qwT persists in the 3 rotating buffers
                    nc.scalar.copy(
                        vt3[:, :, 128 : 128 + L],
                        qwT.rearrange("p (c w) -> p c w", w=L),
                    )
                b_idx += 1
                for half in range(4):
                    ftp = ps_ft.tile([128, 512], f32)
                    for cc in range(4):
                        c = 4 * half + cc
                        nc.tensor.matmul(
                            ftp[:, 128 * cc : 128 * (cc + 1)],
                            X[:, 128 * c : 128 * (c + 1)],
                            mtbd[:],
                            start=True,
                            stop=True,
                        )
                    ft4 = ftp.rearrange("p (cc w) -> p cc w", w=128)
                    dst = vt3[:, 4 * half : 4 * half + 4, 0:128]
                    if half % 2 == 0:
                        nc.scalar.copy(dst, ft4)
                    else:
                        nc.vector.tensor_copy(dst, ft4)

                SCp = ps_sc.tile([128, CW], f32)
                for c in range(NCHUNK):
                    nc.tensor.matmul(
                        SCp[:],
                        vt3[:, c, 0:128],
                        vt3[:, c, 0:CW],
                        start=(c == 0),
                        stop=(c == NCHUNK - 1),
                    )

                junk = small.tile([P, 128], f32)
                sumsq = small.tile([P, 1], f32)
                nc.vector.scalar_tensor_tensor(
                    out=junk[:],
                    in0=SCp[0:P, 0:128],
                    scalar=1.0,
                    in1=eyebd[:],
                    op0=AluOpType.mult,
                    op1=AluOpType.mult,
                    accum_out=sumsq[:],
                )
                lnu = small.tile([P, 1], f32)
                nc.scalar.activation(
                    lnu[:], sumsq[:], mybir.ActivationFunctionType.Ln,
                    bias=epsb[:], scale=1.0 / D,
                )
                rsq = small.tile([P, 1], f32)
                nc.scalar.activation(
                    rsq[:], lnu[:], mybir.ActivationFunctionType.Exp, scale=-0.5
                )
                scoresR = small.tile([P, L], f32)
                nc.scalar.activation(
                    scoresR[:], SCp[0:P, 128 : 128 + L],
                    mybir.ActivationFunctionType.Copy, scale=rsq[:],
                )

                scoreT = ps_sm.tile([L, P], f32, tag="sm")
                nc.tensor.transpose(scoreT[:], scoresR[:], ident[:P, :P])
                smask = small.tile([L, P], f32)
                nc.vector.tensor_add(smask[:], scoreT[:], maskneg[:])
                esc = small.tile([L, P], f32)
                nc.scalar.activation(
                    esc[:], smask[:], mybir.ActivationFunctionType.Exp
                )
                ssum = small.tile([L, R], f32)
                nc.vector.reduce_sum(
                    ssum[:],
                    esc.rearrange("p (r n) -> p r n", r=R),
                    axis=mybir.AxisListType.X,
                )
                rec = small.tile([L, R], f32)
                nc.vector.reciprocal(rec[:], ssum[:])
                alpha = small.tile([L, P], f32)
                nc.vector.tensor_tensor(
                    alpha.rearrange("p (r n) -> p r n", r=R),
                    esc.rearrange("p (r n) -> p r n", r=R),
                    rec.unsqueeze(2).broadcast_to([L, R, NS]),
                    AluOpType.mult,
                )

                alphaT = ps_sm.tile([P, L], f32, tag="sm")
                nc.tensor.transpose(alphaT[:], alpha[:], ident[:L, :L])
                abd = small.tile([P, 128], bf16)
                nc.vector.memset(abd[:, 120:128], 0.0)
                nc.vector.scalar_tensor_tensor(
                    out=abd[:, 0:120].rearrange("p (r l) -> p r l", r=R),
                    in0=alphaT.unsqueeze(1).broadcast_to([P, R, L]),
                    scalar=1.0,
                    in1=diagm.rearrange("p (r l) -> p r l", r=R),
                    op0=AluOpType.mult,
                    op1=AluOpType.mult,
                )
                BTp = ps_sm.tile([128, 128], f32, tag="sm")
                nc.tensor.matmul(BTp[:], mbd[:], abd[:], start=True, stop=True)
                btsb = small.tile([128, 128], bf16)
                nc.vector.tensor_copy(btsb[:], BTp[:])

                h_sb = hpool.tile([R * L, D], bf16)
                for nb in range(4):
                    Hp = ps_h.tile([128, 512], f32)
                    nc.tensor.matmul(
                        Hp[:],
                        btsb[0:P, :],
                        X[:, 512 * nb : 512 * (nb + 1)],
                        start=True,
                        stop=True,
                    )
                    if nb % 2 == 0:
                        nc.scalar.copy(h_sb[:, 512 * nb : 512 * (nb + 1)],
                                       Hp[0 : R * L, :])
                    else:
                        nc.vector.tensor_copy(h_sb[:, 512 * nb : 512 * (nb + 1)],
                                              Hp[0 : R * L, :])

                nc.scalar.dma_start(
                    outT[row0 * L : row0 * L + R * L, :], h_sb[:]
                )

    real_gat = bacc.get_activation_tables
    AF = mybir.ActivationFunctionType

    def gat_pinned(arch):
        out = {}
        for name, fns in real_gat(arch).items():
            if name == "natural_log_exp_and_others":
                out[name] = set(fns)
            else:
                out[name] = {f for f in fns if f not in (AF.Ln, AF.Exp)}
        return out

    bacc.get_activation_tables = gat_pinned
    try:
        if do_compile:
            nc.compile()
    finally:
        bacc.get_activation_tables = real_gat
    return nc


_NC_CACHE = None


def _prep_loT(layer_outputs, embedding):
    loT = np.empty((B * T, NJ, D), dtype=BF)
    loT[:, 0, :] = embedding.reshape(B * T, D).astype(BF)
    loT[:, 1:, :] = (
        layer_outputs.reshape(L, B * T, D).transpose(1, 0, 2).astype(BF)
    )
    return loT


def _make_in_maps(layer_outputs, embedding, queries, key_norm_weight):
    loT = _prep_loT(layer_outputs, embedding)
    consts = _build_consts(queries, key_norm_weight)
    in_maps = []
    for c in range(N_CORES):
        r0 = c * ROWS_PER_CORE
        in_maps.append({
            "loT": loT[r0 : r0 + ROWS_PER_CORE].reshape(ROWS_PER_CORE * NJ, D),
            "qwT": consts["qwT"],
            "mtbd": consts["mtbd"],
            "mbd": consts["mbd"],
            "eyebd": consts["eyebd"],
            "diagm": consts["diagm"],
            "maskneg": consts["maskneg"],
            "ident": consts["ident"],
        })
    return in_maps


def kernel(layer_outputs, embedding, queries, key_norm_weight):
    global _NC_CACHE
    layer_outputs = np.asarray(layer_outputs, dtype=np.float32)
    embedding = np.asarray(embedding, dtype=np.float32)
    queries = np.asarray(queries, dtype=np.float32)
    key_norm_weight = np.asarray(key_norm_weight, dtype=np.float32)

    in_maps = _make_in_maps(layer_outputs, embedding, queries, key_norm_weight)

    if _NC_CACHE is None:
        _NC_CACHE = build_kernel()
    nc = _NC_CACHE

    res = run_bass_kernel_spmd(nc, in_maps, core_ids=list(range(N_CORES)))

    full = np.empty((L, B * T, D), dtype=np.float32)
    for c in range(N_CORES):
        r0 = c * ROWS_PER_CORE
        outT = res.results[c]["outT"].astype(np.float32).reshape(
            ROWS_PER_CORE, L, D
        )
        full[:, r0 : r0 + ROWS_PER_CORE, :] = outT.transpose(1, 0, 2)
    return full.reshape(L, B, T, D)
